# revision 29
# baseline (speedup 1.0000x reference)
"""MoE routed dynamics kernel for Trainium2 (8 NeuronCores, expert-parallel).

Problem: for each row b of a [B, D+A] input, route through one of P=8
two-layer MLPs selected by policy_indices[b]:
    h = relu(x @ W1[p] + b1[p]);  y = h @ W2[p] + b2[p]

Sharding: expert-parallel. Core p owns expert p's weights (resident in
SBUF) and processes exactly the rows routed to expert p. The all-to-all
dispatch keyed on policy_indices happens on the host at shard time
(gather rows by expert, pad to a common capacity C), and the inverse
scatter happens at unshard time.

Device kernel (per core), all activations feature-on-partition so no
transposes are needed anywhere:
    xT   [DA, C]  (DA=576)         input, transposed on host
    hT   [H, C]   = relu(W1.T @ x + b1), H=1024, via PE matmuls
    outT [D, C]   = W2.T @ h + b2,  D=512
Matmuls run as out[M,N] = lhsT.T @ rhs with lhsT = weight chunks in
their natural [K, M] layout and rhs = activation chunks [K, N<=512].

Matmul dtype is bf16 (PSUM accumulation stays fp32): the PE streams
1 col/cycle for both fp32r and bf16, so bf16 costs no matmul time but
halves every load and store byte -- and the kernel's head is bound by
DMA delivery, not compute. Measured rel err ~3.8e-3 (gate is 2e-2).

Layer-1 contraction is K = 576 = 4*128 + 64.  The ragged 64-row tail
is handled by row-packing: the 64 tail rows of x are duplicated into
partitions 64:128 (done on the host), and the tail matmuls for two
adjacent output tiles (m=2j, m=2j+1) run concurrently on row-groups
0:64 / 64:128 of the PE array via tile_position (~1.5 N-cycle spans
instead of two per pair; a 4x K=32 variant using row-group (96,0)
hangs the device -- quadrant-3 HW bug -- so stay with pairs).

Per-chunk outputs accumulate into one fused [128, 4*nl] y tile and
store as a single DMA (partition-first 3D access pattern); the final
chunk keeps per-d stores on the idle Sync ring to minimize the tail
chain.

Schedule shape (learned from NTFF profiles):
- ~6-7us fixed framework preamble before any kernel op, ~9us fixed
  drain + barrier + semaphore-sweep epilogue after the last store.
- Per-DMA completion latency at kernel start is ~3-5us (8 cores hammer
  HBM simultaneously), so chunk-0 x tiles ship FUSED with the W1 tiles
  they are consumed with (one DMA each), all loads ride the Sync HWDGE
  ring in first-use order, and ~24 junk matmuls on a memset scratch
  warm the PE HAM clock gate (1.2 -> 2.4 GHz) while the first tiles
  are in flight. The smallest column chunk runs first (smaller critical
  tiles), the second-smallest last (short tail); x prefetch is capped
  at 2 chunks in flight (xpool bufs) so mid-kernel loads don't crowd
  the wire while W1/W2/x1 are still landing.
- Compute is software-pipelined one chunk ahead (L1 of chunk c+1
  before L2 of chunk c) so W2's arrival and each x chunk hide behind
  matmul streaming.
- Output stores issue from Scalar's HWDGE ring (Sync is busy with
  loads), except the last chunk where Sync is idle and Scalar would
  serialize the final bias-adds and stores.
"""

import math
import os

import numpy as np

_B = 16384
_P = 8
_D = 512
_A = 64
_H = 1024
_DA = _D + _A   # 576
_KF = 4         # full 128-row K chunks of layer 1
_N_CORES = 8

_MM_DTYPE = os.environ.get("MM_DTYPE", "bfloat16")
# layer-1 ragged-tail handling: "quad" = 4x K=32 row-tiled matmuls,
# "pair" = 2x K=64, "serial" = no tile_position (debug)
_TAIL_MODE = os.environ.get("TAIL_MODE", "pair")

_kernel_cache: dict = {}


def _n_chunks(C: int):
    """Column chunking: all chunks >= 256 (fp32r full-rate), <= 512 (one
    PSUM bank). Smallest chunk last (short kernel tail), second-smallest
    first (fast DMA-paced warm-up)."""
    sizes = []
    rem = C
    while rem > 1023:
        sizes.append(512)
        rem -= 512
    if rem >= 768:
        sizes.extend([512, rem - 512])
    else:
        sizes.extend([rem - 256, 256])
    # Smallest chunk FIRST: the head is DMA-latency bound, and a small
    # first chunk shrinks the critical fused [x0|W1] tiles. Second-
    # smallest last keeps the kernel tail short. Big chunks in between.
    asc = sorted(sizes)
    order = [asc[0]] + sorted(asc[2:], reverse=True) + [asc[1]]
    out = []
    n0 = 0
    for nl in order:
        out.append((n0, nl))
        n0 += nl
    assert n0 == C and all(256 <= nl <= 512 for _, nl in out), (C, out)
    return out


def _build_bass(C: int):
    import concourse.bacc as bacc
    import concourse.mybir as mybir
    from concourse.tile import TileContext

    fp32 = mybir.dt.float32
    mmdt = getattr(mybir.dt, _MM_DTYPE)
    act = mybir.ActivationFunctionType

    n_chunks = _n_chunks(C)
    mh = _H // 128        # 8 output tiles of layer 1
    md = _D // 128        # 4 output tiles of layer 2
    kh = _H // 128        # 8 K chunks of layer 2

    nl_0 = n_chunks[0][1]
    nc = bacc.Bacc()
    # x: 4 full K chunks + the 64-row tail duplicated into both halves.
    # Chunk-0 columns ship fused with the weights they are consumed with
    # (xw below), so xT only carries chunks 1..last.
    xT = nc.declare_dram_parameter("xT", [_KF + 1, 128, C - nl_0], mmdt, isOutput=False)
    # xw[k] = [chunk-0 x tile k | W1 half/tail needed with it]:
    #   k<4:  [x0_k (nl_0) | W1[k] cols 0:512]
    #   k=4:  [x0_tail-dup (nl_0) | W1 tail pairs (rows 0:64 = tile 2j,
    #          rows 64:128 = tile 2j+1)]
    # One DMA delivers both, halving the head's completion-latency chain.
    xw = nc.declare_dram_parameter("xw", [_KF + 1, 128, nl_0 + _H // 2], mmdt, isOutput=False)
    # W1 cols 512:1024 (tiles m=4..7), needed a few us later
    w1f = nc.declare_dram_parameter("w1f", [_KF, 128, _H // 2], mmdt, isOutput=False)
    w2 = nc.declare_dram_parameter("w2", [kh, 128, _D], mmdt, isOutput=False)
    # biases packed together: cols 0:mh = b1 tiles, mh:mh+md = b2 tiles
    b12 = nc.declare_dram_parameter("b12", [128, mh + md], fp32, isOutput=False)
    # Output stays in the matmul dtype (bf16 halves store traffic; host
    # upcasts). fp32 PSUM -> bf16 rounding adds ~2e-4 relative error.
    # Layout [128, md*C]: per partition, md blocks of C columns, so a
    # whole chunk (all md row-tiles) stores as ONE DMA with a regular
    # partition-first 3D access pattern.
    outT = nc.declare_dram_parameter("outT", [128, md * C], mmdt, isOutput=True)

    with TileContext(nc) as tc:
        with (
            tc.tile_pool(name="wpool", bufs=1) as wpool,
            tc.tile_pool(name="xpool", bufs=2) as xpool,
            tc.tile_pool(name="hpool", bufs=3) as hpool,
            tc.tile_pool(name="ypool", bufs=3) as ypool,
            tc.tile_pool(name="ps1", bufs=4, space="PSUM") as ps1,
            tc.tile_pool(name="ps2", bufs=4, space="PSUM") as ps2,
        ):
            xw_tiles = [
                wpool.tile([128, nl_0 + _H // 2], mmdt, name=f"xw_{k}", tag=f"xw_{k}")
                for k in range(_KF + 1)
            ]
            w1_tiles = [
                wpool.tile([128, _H // 2], mmdt, name=f"w1_{k}", tag=f"w1_{k}")
                for k in range(_KF)
            ]

            def w1s(k, m):
                if m < mh // 2:
                    return xw_tiles[k][:, nl_0 + m * 128 : nl_0 + (m + 1) * 128]
                return w1_tiles[k][:, (m - mh // 2) * 128 : (m - mh // 2 + 1) * 128]

            w2_tiles = [
                wpool.tile([128, _D], mmdt, name=f"w2_{k}", tag=f"w2_{k}")
                for k in range(kh)
            ]

            def w2s(k, d):
                return w2_tiles[k][:, d * 128 : (d + 1) * 128]

            # --- DMA issue plan ------------------------------------------
            # All loads share the Sync HWDGE ring so HBM delivery follows
            # issue order exactly (a second load ring would round-robin
            # packets and delay the critical head tiles). Order = order of
            # first compute use: fused [x0_k | W1_k] tiles, W1 cols
            # 512:1024, x chunk 1, W2 (needed only when L2(c0) runs, after
            # L1(c1)), then x2..x4.
            for k in range(_KF + 1):
                nc.sync.dma_start(out=xw_tiles[k][:, :], in_=xw[k, :, :])
            b12_sb = wpool.tile([128, mh + md], fp32, tag="b12")
            nc.scalar.dma_start(out=b12_sb[:], in_=b12[:, :])

            x_first = [xw_tiles[k][:, :nl_0] for k in range(_KF + 1)]
            w1t_sb = xw_tiles[_KF][:, nl_0:]

            def dma_x(n0, nl):
                tiles = []
                for k in range(_KF + 1):
                    t = xpool.tile([128, nl], mmdt, tag=f"x_{k}")
                    nc.sync.dma_start(out=t[:, :], in_=xT[k, :, n0 - nl_0 : n0 - nl_0 + nl])
                    tiles.append(t)
                return tiles

            for k in range(_KF):
                nc.sync.dma_start(out=w1_tiles[k][:, :], in_=w1f[k, :, :])
            x_all = [x_first, dma_x(*n_chunks[1])]
            for k in range(kh):
                nc.sync.dma_start(out=w2_tiles[k][:, :], in_=w2[k, :, :])
            x_all += [dma_x(n0, nl) for n0, nl in n_chunks[2:]]

            # --- PE warm-up ----------------------------------------------
            # The PE HAM clock gate only reaches 8/8 (2.4 GHz) after ~3.4us
            # of sustained activity. Real matmuls can't start until the
            # first x/w tiles land (~4us after the preamble), so burn that
            # DMA-wait on junk matmuls over a memset scratch tile: by the
            # time data arrives the PE is already at full clock.
            warm = wpool.tile([128, 256], mmdt, tag="warm")
            nc.vector.memset(warm[:, :], 0)
            # Scratch PSUM from the ps2 pool (first real ps2 use is ~15us
            # later, so the WAW dep on the warm-up group never stalls).
            # 23 x N=256 at the cold 1.2 GHz clock ~= 5us of PE activity,
            # which covers the gap until the first x/w tiles land (~12.5us:
            # ~7us preamble+issue plus ~4-5us DMA completion latency while
            # all 8 cores hammer HBM at once).
            wps = ps2.tile([128, 256], fp32, tag="ps2")
            for i in range(24):
                nc.tensor.matmul(
                    wps[:, :], warm[:, 0:128], warm[:, :],
                    start=(i == 0), stop=(i == 23),
                )

            # --- compute -------------------------------------------------
            def l1(ci):
                n0, nl = n_chunks[ci]
                x_sb = x_all[ci]
                h_sb = [None] * mh
                for j in range(mh // 2):
                    ma, mb = 2 * j, 2 * j + 1
                    psa = ps1.tile([128, nl], fp32, tag="ps1")
                    for k in range(_KF):
                        nc.tensor.matmul(
                            psa[:, :], w1s(k, ma), x_sb[k][:, :],
                            start=(k == 0), stop=False,
                        )
                    psb = ps1.tile([128, nl], fp32, tag="ps1")
                    for k in range(_KF):
                        nc.tensor.matmul(
                            psb[:, :], w1s(k, mb), x_sb[k][:, :],
                            start=(k == 0), stop=False,
                        )
                    # Ragged K=64 tails for tiles (2j, 2j+1): adjacent in
                    # the queue, on disjoint PE row-groups so they overlap.
                    jc = slice(j * 128, (j + 1) * 128)
                    if _TAIL_MODE == "quad":
                        # 4x K=32 on row-groups 0/32/64/96 (K=32 row tiling
                        # measured closest to full concurrency)
                        for r, ps_t, st in (
                            (0, psa, False), (32, psa, True),
                            (64, psb, False), (96, psb, True),
                        ):
                            nc.tensor.matmul(
                                ps_t[:, :],
                                w1t_sb[r : r + 32, jc],
                                x_sb[_KF][r : r + 32, :],
                                start=False, stop=st, tile_position=(r, 0),
                            )
                    else:
                        tp = _TAIL_MODE == "pair"
                        nc.tensor.matmul(
                            psa[:, :], w1t_sb[0:64, jc], x_sb[_KF][0:64, :],
                            start=False, stop=True,
                            tile_position=(0, 0) if tp else None,
                        )
                        nc.tensor.matmul(
                            psb[:, :], w1t_sb[64:128, jc], x_sb[_KF][64:128, :],
                            start=False, stop=True,
                            tile_position=(64, 0) if tp else None,
                        )
                    for m, ps in ((ma, psa), (mb, psb)):
                        ht = hpool.tile([128, nl], mmdt, tag=f"h_{m}")
                        nc.scalar.activation(
                            ht[:], ps[:], act.Relu, bias=b12_sb[:, m : m + 1]
                        )
                        h_sb[m] = ht
                return h_sb

            outT_3d = outT[:, :].rearrange("p (d c) -> p d c", d=md)

            def l2(ci, h_sb):
                n0, nl = n_chunks[ci]
                last = ci == len(n_chunks) - 1
                y4 = None if last else ypool.tile([128, md * nl], mmdt, tag="y4")
                for d in range(md):
                    ps = ps2.tile([128, nl], fp32, tag="ps2")
                    for m in range(mh):
                        nc.tensor.matmul(
                            ps[:, :], w2s(m, d), h_sb[m][:, :],
                            start=(m == 0), stop=(m == mh - 1),
                        )
                    if not last:
                        # Bias-add lands in the d-th block of a fused y
                        # tile; the whole chunk stores as one DMA below.
                        nc.vector.tensor_scalar_add(
                            y4[:, d * nl : (d + 1) * nl], ps[:, :],
                            b12_sb[:, mh + d : mh + d + 1],
                        )
                        continue
                    # Final chunk: per-d stores on the now-idle Sync ring,
                    # bias-adds alternating DVE/ACT, so the last outputs
                    # drain with minimum serialization.
                    yt = ypool.tile([128, nl], mmdt, tag="y")
                    if d % 2 == 1:
                        nc.scalar.activation(
                            yt[:, :], ps[:, :], act.Identity,
                            bias=b12_sb[:, mh + d : mh + d + 1],
                        )
                    else:
                        nc.vector.tensor_scalar_add(
                            yt[:, :], ps[:, :], b12_sb[:, mh + d : mh + d + 1]
                        )
                    nc.sync.dma_start(
                        out=outT_3d[:, d, n0 : n0 + nl], in_=yt[:, :]
                    )
                if not last:
                    y4v = y4[:, :].rearrange("p (d c) -> p d c", d=md)
                    nc.scalar.dma_start(
                        out=outT_3d[:, :, n0 : n0 + nl], in_=y4v[:, :, :]
                    )

            # Software-pipelined by one chunk: L1 of chunk c+1 runs before
            # L2 of chunk c, so W2's arrival (2 MB after W1+x0+x1) and each
            # x chunk hide behind compute.
            nch = len(n_chunks)
            h_prev = l1(0)
            for ci in range(1, nch):
                h_cur = l1(ci)
                l2(ci - 1, h_prev)
                h_prev = h_cur
            l2(nch - 1, h_prev)

    nc.compile()
    return nc


def _get_bass(C: int):
    nc = _kernel_cache.get(C)
    if nc is None:
        nc = _build_bass(C)
        _kernel_cache[C] = nc
    return nc


def _mm_np(a):
    """Cast a float32 array to the numpy dtype matching _MM_DTYPE."""
    if _MM_DTYPE == "bfloat16":
        import ml_dtypes

        return np.ascontiguousarray(a.astype(ml_dtypes.bfloat16))
    return np.ascontiguousarray(a)


def _prepare_in_maps(latents, actions, policy_indices, W1, b1, W2, b2):
    """Expert-parallel dispatch: returns (in_maps, C, order, offs, counts)."""
    latents = np.asarray(latents, dtype=np.float32)
    actions = np.asarray(actions, dtype=np.float32)
    pi = np.asarray(policy_indices).astype(np.int64)
    W1 = np.asarray(W1, dtype=np.float32)
    b1 = np.asarray(b1, dtype=np.float32)
    W2 = np.asarray(W2, dtype=np.float32)
    b2 = np.asarray(b2, dtype=np.float32)

    B = latents.shape[0]
    counts = np.bincount(pi, minlength=_P)
    order = np.argsort(pi, kind="stable")
    offs = np.concatenate(([0], np.cumsum(counts)))

    # Per-core capacity: smallest multiple of 64 >= max rows per expert
    # (>= 1536 so the chunking always yields >=256-wide chunks).
    C = max(1536, int(math.ceil(counts.max() / 64)) * 64)

    x = np.empty((B, _DA), dtype=np.float32)
    x[:, :_D] = latents
    x[:, _D:] = actions
    x_sorted = x[order]

    mh = _H // 128
    md = _D // 128
    nl_0 = _n_chunks(C)[0][1]
    in_maps = []
    for p in range(_P):
        cp = counts[p]
        xp = np.zeros((_KF + 1, 128, C), dtype=np.float32)
        xs = x_sorted[offs[p] : offs[p + 1]].T          # [576, cp]
        xp[:_KF, :, :cp] = xs[: 4 * 128].reshape(_KF, 128, cp)
        xp[_KF, 0:64, :cp] = xs[4 * 128 :]
        xp[_KF, 64:128, :cp] = xs[4 * 128 :]            # duplicated tail
        w1p = W1[p]                                     # [576, 1024]
        w1fp = w1p[: 4 * 128].reshape(_KF, 128, _H)
        w1tp = np.zeros((128, (mh // 2) * 128), dtype=np.float32)
        tail = w1p[4 * 128 :]                           # [64, 1024]
        for j in range(mh // 2):
            w1tp[0:64, j * 128 : (j + 1) * 128] = tail[:, (2 * j) * 128 : (2 * j + 1) * 128]
            w1tp[64:128, j * 128 : (j + 1) * 128] = tail[:, (2 * j + 1) * 128 : (2 * j + 2) * 128]
        # Fused [chunk-0 x | first-needed W1] tiles (one DMA each on device)
        xwp = np.empty((_KF + 1, 128, nl_0 + _H // 2), dtype=np.float32)
        xwp[:, :, :nl_0] = xp[:, :, :nl_0]
        xwp[:_KF, :, nl_0:] = w1fp[:, :, : _H // 2]
        xwp[_KF, :, nl_0:] = w1tp
        b12p = np.concatenate(
            [b1[p].reshape(mh, 128).T, b2[p].reshape(md, 128).T], axis=1
        )
        in_maps.append(
            {
                "xT": _mm_np(xp[:, :, nl_0:]),
                "xw": _mm_np(xwp),
                "w1f": _mm_np(w1fp[:, :, _H // 2 :]),
                "w2": _mm_np(W2[p].reshape(_H // 128, 128, _D)),
                "b12": np.ascontiguousarray(b12p),
            }
        )
    return in_maps, C, order, offs, counts


def kernel(latents, actions, policy_indices, W1, b1, W2, b2):
    from concourse.bass_utils import run_bass_kernel_spmd

    in_maps, C, order, offs, counts = _prepare_in_maps(
        latents, actions, policy_indices, W1, b1, W2, b2
    )
    nc = _get_bass(C)
    results = run_bass_kernel_spmd(nc, in_maps, list(range(_N_CORES))).results

    B = np.asarray(latents).shape[0]
    md = _D // 128
    out = np.empty((B, _D), dtype=np.float32)
    for p in range(_P):
        yT = (
            np.asarray(results[p]["outT"], dtype=np.float32)
            .reshape(128, md, C)
            .transpose(1, 0, 2)
            .reshape(_D, C)
        )
        out[order[offs[p] : offs[p + 1]]] = yT[:, : counts[p]].T
    return out


# revision 30
# speedup vs baseline: 1.0335x; 1.0335x over previous
"""MoE routed dynamics kernel for Trainium2 (8 NeuronCores, expert-parallel).

Problem: for each row b of a [B, D+A] input, route through one of P=8
two-layer MLPs selected by policy_indices[b]:
    h = relu(x @ W1[p] + b1[p]);  y = h @ W2[p] + b2[p]

Sharding: expert-parallel. Core p owns expert p's weights (resident in
SBUF) and processes exactly the rows routed to expert p. The all-to-all
dispatch keyed on policy_indices happens on the host at shard time
(gather rows by expert, pad to a common capacity C), and the inverse
scatter happens at unshard time.

Device kernel (per core), all activations feature-on-partition so no
transposes are needed anywhere:
    xT   [DA, C]  (DA=576)         input, transposed on host
    hT   [H, C]   = relu(W1.T @ x + b1), H=1024, via PE matmuls
    outT [D, C]   = W2.T @ h + b2,  D=512
Matmuls run as out[M,N] = lhsT.T @ rhs with lhsT = weight chunks in
their natural [K, M] layout and rhs = activation chunks [K, N<=512].

Matmul dtype is bf16 (PSUM accumulation stays fp32): the PE streams
1 col/cycle for both fp32r and bf16, so bf16 costs no matmul time but
halves every load and store byte -- and the kernel's head is bound by
DMA delivery, not compute. Measured rel err ~3.8e-3 (gate is 2e-2).

Layer-1 contraction is K = 576 = 4*128 + 64.  The ragged 64-row tail
is handled by row-packing: the 64 tail rows of x are duplicated into
partitions 64:128 (done on the host), and the tail matmuls for two
adjacent output tiles (m=2j, m=2j+1) run concurrently on row-groups
0:64 / 64:128 of the PE array via tile_position (~1.5 N-cycle spans
instead of two per pair; a 4x K=32 variant using row-group (96,0)
hangs the device -- quadrant-3 HW bug -- so stay with pairs).

Per-chunk outputs accumulate into one fused [128, 4*nl] y tile and
store as a single DMA (partition-first 3D access pattern); the final
chunk keeps per-d stores on the idle Sync ring to minimize the tail
chain.

Schedule shape (learned from NTFF profiles):
- ~6-7us fixed framework preamble before any kernel op, ~9us fixed
  drain + barrier + semaphore-sweep epilogue after the last store.
- Per-DMA completion latency at kernel start is ~3-5us (8 cores hammer
  HBM simultaneously), so chunk-0 x tiles ship FUSED with the W1 tiles
  they are consumed with (one DMA each), all loads ride the Sync HWDGE
  ring in first-use order, and ~24 junk matmuls on a memset scratch
  warm the PE HAM clock gate (1.2 -> 2.4 GHz) while the first tiles
  are in flight. The smallest column chunk runs first (smaller critical
  tiles), the second-smallest last (short tail); x prefetch is capped
  at 2 chunks in flight (xpool bufs) so mid-kernel loads don't crowd
  the wire while W1/W2/x1 are still landing.
- Compute is software-pipelined one chunk ahead (L1 of chunk c+1
  before L2 of chunk c) so W2's arrival and each x chunk hide behind
  matmul streaming.
- Output stores issue from Scalar's HWDGE ring (Sync is busy with
  loads), except the last chunk where Sync is idle and Scalar would
  serialize the final bias-adds and stores.
"""

import math
import os

import numpy as np

_B = 16384
_P = 8
_D = 512
_A = 64
_H = 1024
_DA = _D + _A   # 576
_KF = 4         # full 128-row K chunks of layer 1
_N_CORES = 8

_MM_DTYPE = os.environ.get("MM_DTYPE", "bfloat16")
# layer-1 ragged-tail handling: "quad" = 4x K=32 row-tiled matmuls,
# "pair" = 2x K=64, "serial" = no tile_position (debug)
_TAIL_MODE = os.environ.get("TAIL_MODE", "pair")

_kernel_cache: dict = {}


def _n_chunks(C: int):
    """Column chunking: all chunks >= 256 (fp32r full-rate), <= 512 (one
    PSUM bank). Smallest chunk last (short kernel tail), second-smallest
    first (fast DMA-paced warm-up)."""
    sizes = []
    rem = C
    while rem > 1023:
        sizes.append(512)
        rem -= 512
    if rem >= 768:
        sizes.extend([512, rem - 512])
    else:
        sizes.extend([rem - 256, 256])
    # Smallest chunk FIRST: the head is DMA-latency bound, and a small
    # first chunk shrinks the critical fused [x0|W1] tiles. Second-
    # smallest last keeps the kernel tail short. Big chunks in between.
    asc = sorted(sizes)
    order = [asc[1]] + sorted(asc[2:], reverse=True) + [asc[0]]
    out = []
    n0 = 0
    for nl in order:
        out.append((n0, nl))
        n0 += nl
    assert n0 == C and all(256 <= nl <= 512 for _, nl in out), (C, out)
    return out


def _build_bass(C: int):
    import concourse.bacc as bacc
    import concourse.mybir as mybir
    from concourse.tile import TileContext

    fp32 = mybir.dt.float32
    mmdt = getattr(mybir.dt, _MM_DTYPE)
    act = mybir.ActivationFunctionType

    n_chunks = _n_chunks(C)
    mh = _H // 128        # 8 output tiles of layer 1
    md = _D // 128        # 4 output tiles of layer 2
    kh = _H // 128        # 8 K chunks of layer 2

    nl_0 = n_chunks[0][1]
    nc = bacc.Bacc()
    # x: 4 full K chunks + the 64-row tail duplicated into both halves.
    # Chunk-0 columns ship fused with the weights they are consumed with
    # (xw below), so xT only carries chunks 1..last.
    xT = nc.declare_dram_parameter("xT", [_KF + 1, 128, C - nl_0], mmdt, isOutput=False)
    # xw[k] = [chunk-0 x tile k | W1 half/tail needed with it]:
    #   k<4:  [x0_k (nl_0) | W1[k] cols 0:512]
    #   k=4:  [x0_tail-dup (nl_0) | W1 tail pairs (rows 0:64 = tile 2j,
    #          rows 64:128 = tile 2j+1)]
    # One DMA delivers both, halving the head's completion-latency chain.
    xw = nc.declare_dram_parameter("xw", [_KF + 1, 128, nl_0 + _H // 2], mmdt, isOutput=False)
    # W1 cols 512:1024 (tiles m=4..7), needed a few us later
    w1f = nc.declare_dram_parameter("w1f", [_KF, 128, _H // 2], mmdt, isOutput=False)
    w2 = nc.declare_dram_parameter("w2", [kh, 128, _D], mmdt, isOutput=False)
    # biases packed together: cols 0:mh = b1 tiles, mh:mh+md = b2 tiles
    b12 = nc.declare_dram_parameter("b12", [128, mh + md], fp32, isOutput=False)
    # Output stays in the matmul dtype (bf16 halves store traffic; host
    # upcasts). fp32 PSUM -> bf16 rounding adds ~2e-4 relative error.
    # Layout [128, md*C]: per partition, md blocks of C columns, so a
    # whole chunk (all md row-tiles) stores as ONE DMA with a regular
    # partition-first 3D access pattern.
    outT = nc.declare_dram_parameter("outT", [128, md * C], mmdt, isOutput=True)

    with TileContext(nc) as tc:
        with (
            tc.tile_pool(name="wpool", bufs=1) as wpool,
            tc.tile_pool(name="xpool", bufs=2) as xpool,
            tc.tile_pool(name="hpool", bufs=3) as hpool,
            tc.tile_pool(name="ypool", bufs=3) as ypool,
            tc.tile_pool(name="ps1", bufs=4, space="PSUM") as ps1,
            tc.tile_pool(name="ps2", bufs=4, space="PSUM") as ps2,
        ):
            xw_tiles = [
                wpool.tile([128, nl_0 + _H // 2], mmdt, name=f"xw_{k}", tag=f"xw_{k}")
                for k in range(_KF + 1)
            ]
            w1_tiles = [
                wpool.tile([128, _H // 2], mmdt, name=f"w1_{k}", tag=f"w1_{k}")
                for k in range(_KF)
            ]

            def w1s(k, m):
                if m < mh // 2:
                    return xw_tiles[k][:, nl_0 + m * 128 : nl_0 + (m + 1) * 128]
                return w1_tiles[k][:, (m - mh // 2) * 128 : (m - mh // 2 + 1) * 128]

            w2_tiles = [
                wpool.tile([128, _D], mmdt, name=f"w2_{k}", tag=f"w2_{k}")
                for k in range(kh)
            ]

            def w2s(k, d):
                return w2_tiles[k][:, d * 128 : (d + 1) * 128]

            # --- DMA issue plan ------------------------------------------
            # All loads share the Sync HWDGE ring so HBM delivery follows
            # issue order exactly (a second load ring would round-robin
            # packets and delay the critical head tiles). Order = order of
            # first compute use: fused [x0_k | W1_k] tiles, W1 cols
            # 512:1024, x chunk 1, W2 (needed only when L2(c0) runs, after
            # L1(c1)), then x2..x4.
            for k in range(_KF + 1):
                nc.sync.dma_start(out=xw_tiles[k][:, :], in_=xw[k, :, :])
            b12_sb = wpool.tile([128, mh + md], fp32, tag="b12")
            nc.scalar.dma_start(out=b12_sb[:], in_=b12[:, :])

            x_first = [xw_tiles[k][:, :nl_0] for k in range(_KF + 1)]
            w1t_sb = xw_tiles[_KF][:, nl_0:]

            def dma_x(n0, nl):
                tiles = []
                for k in range(_KF + 1):
                    t = xpool.tile([128, nl], mmdt, tag=f"x_{k}")
                    nc.sync.dma_start(out=t[:, :], in_=xT[k, :, n0 - nl_0 : n0 - nl_0 + nl])
                    tiles.append(t)
                return tiles

            for k in range(_KF):
                nc.sync.dma_start(out=w1_tiles[k][:, :], in_=w1f[k, :, :])
            x_all = [x_first, dma_x(*n_chunks[1])]
            for k in range(kh):
                nc.sync.dma_start(out=w2_tiles[k][:, :], in_=w2[k, :, :])
            x_all += [dma_x(n0, nl) for n0, nl in n_chunks[2:]]

            # --- PE warm-up ----------------------------------------------
            # The PE HAM clock gate only reaches 8/8 (2.4 GHz) after ~3.4us
            # of sustained activity. Real matmuls can't start until the
            # first x/w tiles land (~4us after the preamble), so burn that
            # DMA-wait on junk matmuls over a memset scratch tile: by the
            # time data arrives the PE is already at full clock.
            warm = wpool.tile([128, 256], mmdt, tag="warm")
            nc.vector.memset(warm[:, :], 0)
            # Scratch PSUM from the ps2 pool (first real ps2 use is ~15us
            # later, so the WAW dep on the warm-up group never stalls).
            # 23 x N=256 at the cold 1.2 GHz clock ~= 5us of PE activity,
            # which covers the gap until the first x/w tiles land (~12.5us:
            # ~7us preamble+issue plus ~4-5us DMA completion latency while
            # all 8 cores hammer HBM at once).
            wps = ps2.tile([128, 256], fp32, tag="ps2")
            for i in range(24):
                nc.tensor.matmul(
                    wps[:, :], warm[:, 0:128], warm[:, :],
                    start=(i == 0), stop=(i == 23),
                )

            # --- compute -------------------------------------------------
            def l1(ci):
                n0, nl = n_chunks[ci]
                x_sb = x_all[ci]
                h_sb = [None] * mh
                for j in range(mh // 2):
                    ma, mb = 2 * j, 2 * j + 1
                    psa = ps1.tile([128, nl], fp32, tag="ps1")
                    for k in range(_KF):
                        nc.tensor.matmul(
                            psa[:, :], w1s(k, ma), x_sb[k][:, :],
                            start=(k == 0), stop=False,
                        )
                    psb = ps1.tile([128, nl], fp32, tag="ps1")
                    for k in range(_KF):
                        nc.tensor.matmul(
                            psb[:, :], w1s(k, mb), x_sb[k][:, :],
                            start=(k == 0), stop=False,
                        )
                    # Ragged K=64 tails for tiles (2j, 2j+1): adjacent in
                    # the queue, on disjoint PE row-groups so they overlap.
                    jc = slice(j * 128, (j + 1) * 128)
                    if _TAIL_MODE == "quad":
                        # 4x K=32 on row-groups 0/32/64/96 (K=32 row tiling
                        # measured closest to full concurrency)
                        for r, ps_t, st in (
                            (0, psa, False), (32, psa, True),
                            (64, psb, False), (96, psb, True),
                        ):
                            nc.tensor.matmul(
                                ps_t[:, :],
                                w1t_sb[r : r + 32, jc],
                                x_sb[_KF][r : r + 32, :],
                                start=False, stop=st, tile_position=(r, 0),
                            )
                    else:
                        tp = _TAIL_MODE == "pair"
                        nc.tensor.matmul(
                            psa[:, :], w1t_sb[0:64, jc], x_sb[_KF][0:64, :],
                            start=False, stop=True,
                            tile_position=(0, 0) if tp else None,
                        )
                        nc.tensor.matmul(
                            psb[:, :], w1t_sb[64:128, jc], x_sb[_KF][64:128, :],
                            start=False, stop=True,
                            tile_position=(64, 0) if tp else None,
                        )
                    for m, ps in ((ma, psa), (mb, psb)):
                        ht = hpool.tile([128, nl], mmdt, tag=f"h_{m}")
                        nc.scalar.activation(
                            ht[:], ps[:], act.Relu, bias=b12_sb[:, m : m + 1]
                        )
                        h_sb[m] = ht
                return h_sb

            outT_3d = outT[:, :].rearrange("p (d c) -> p d c", d=md)

            def l2(ci, h_sb):
                n0, nl = n_chunks[ci]
                last = ci == len(n_chunks) - 1
                y4 = None if last else ypool.tile([128, md * nl], mmdt, tag="y4")
                for d in range(md):
                    ps = ps2.tile([128, nl], fp32, tag="ps2")
                    for m in range(mh):
                        nc.tensor.matmul(
                            ps[:, :], w2s(m, d), h_sb[m][:, :],
                            start=(m == 0), stop=(m == mh - 1),
                        )
                    if not last:
                        # Bias-add lands in the d-th block of a fused y
                        # tile; the whole chunk stores as one DMA below.
                        nc.vector.tensor_scalar_add(
                            y4[:, d * nl : (d + 1) * nl], ps[:, :],
                            b12_sb[:, mh + d : mh + d + 1],
                        )
                        continue
                    # Final chunk: per-d stores on the now-idle Sync ring,
                    # bias-adds alternating DVE/ACT, so the last outputs
                    # drain with minimum serialization.
                    yt = ypool.tile([128, nl], mmdt, tag="y")
                    if d % 2 == 1:
                        nc.scalar.activation(
                            yt[:, :], ps[:, :], act.Identity,
                            bias=b12_sb[:, mh + d : mh + d + 1],
                        )
                    else:
                        nc.vector.tensor_scalar_add(
                            yt[:, :], ps[:, :], b12_sb[:, mh + d : mh + d + 1]
                        )
                    nc.sync.dma_start(
                        out=outT_3d[:, d, n0 : n0 + nl], in_=yt[:, :]
                    )
                if not last:
                    # Issue from Sync, NOT Scalar: in the Scalar FIFO this
                    # store (which waits for this chunk's last bias-add)
                    # would block the NEXT chunk's relus, stalling the PE
                    # ~0.5us per chunk on ps1 recycling. Sync's load queue
                    # is drained by the time these stores become ready.
                    y4v = y4[:, :].rearrange("p (d c) -> p d c", d=md)
                    nc.sync.dma_start(
                        out=outT_3d[:, :, n0 : n0 + nl], in_=y4v[:, :, :]
                    )

            # Software-pipelined by one chunk: L1 of chunk c+1 runs before
            # L2 of chunk c, so W2's arrival (2 MB after W1+x0+x1) and each
            # x chunk hide behind compute.
            nch = len(n_chunks)
            h_prev = l1(0)
            for ci in range(1, nch):
                h_cur = l1(ci)
                l2(ci - 1, h_prev)
                h_prev = h_cur
            l2(nch - 1, h_prev)

    nc.compile()
    return nc


def _get_bass(C: int):
    nc = _kernel_cache.get(C)
    if nc is None:
        nc = _build_bass(C)
        _kernel_cache[C] = nc
    return nc


def _mm_np(a):
    """Cast a float32 array to the numpy dtype matching _MM_DTYPE."""
    if _MM_DTYPE == "bfloat16":
        import ml_dtypes

        return np.ascontiguousarray(a.astype(ml_dtypes.bfloat16))
    return np.ascontiguousarray(a)


def _prepare_in_maps(latents, actions, policy_indices, W1, b1, W2, b2):
    """Expert-parallel dispatch: returns (in_maps, C, order, offs, counts)."""
    latents = np.asarray(latents, dtype=np.float32)
    actions = np.asarray(actions, dtype=np.float32)
    pi = np.asarray(policy_indices).astype(np.int64)
    W1 = np.asarray(W1, dtype=np.float32)
    b1 = np.asarray(b1, dtype=np.float32)
    W2 = np.asarray(W2, dtype=np.float32)
    b2 = np.asarray(b2, dtype=np.float32)

    B = latents.shape[0]
    counts = np.bincount(pi, minlength=_P)
    order = np.argsort(pi, kind="stable")
    offs = np.concatenate(([0], np.cumsum(counts)))

    # Per-core capacity: smallest multiple of 64 >= max rows per expert
    # (>= 1536 so the chunking always yields >=256-wide chunks).
    C = max(1536, int(math.ceil(counts.max() / 64)) * 64)

    x = np.empty((B, _DA), dtype=np.float32)
    x[:, :_D] = latents
    x[:, _D:] = actions
    x_sorted = x[order]

    mh = _H // 128
    md = _D // 128
    nl_0 = _n_chunks(C)[0][1]
    in_maps = []
    for p in range(_P):
        cp = counts[p]
        xp = np.zeros((_KF + 1, 128, C), dtype=np.float32)
        xs = x_sorted[offs[p] : offs[p + 1]].T          # [576, cp]
        xp[:_KF, :, :cp] = xs[: 4 * 128].reshape(_KF, 128, cp)
        xp[_KF, 0:64, :cp] = xs[4 * 128 :]
        xp[_KF, 64:128, :cp] = xs[4 * 128 :]            # duplicated tail
        w1p = W1[p]                                     # [576, 1024]
        w1fp = w1p[: 4 * 128].reshape(_KF, 128, _H)
        w1tp = np.zeros((128, (mh // 2) * 128), dtype=np.float32)
        tail = w1p[4 * 128 :]                           # [64, 1024]
        for j in range(mh // 2):
            w1tp[0:64, j * 128 : (j + 1) * 128] = tail[:, (2 * j) * 128 : (2 * j + 1) * 128]
            w1tp[64:128, j * 128 : (j + 1) * 128] = tail[:, (2 * j + 1) * 128 : (2 * j + 2) * 128]
        # Fused [chunk-0 x | first-needed W1] tiles (one DMA each on device)
        xwp = np.empty((_KF + 1, 128, nl_0 + _H // 2), dtype=np.float32)
        xwp[:, :, :nl_0] = xp[:, :, :nl_0]
        xwp[:_KF, :, nl_0:] = w1fp[:, :, : _H // 2]
        xwp[_KF, :, nl_0:] = w1tp
        b12p = np.concatenate(
            [b1[p].reshape(mh, 128).T, b2[p].reshape(md, 128).T], axis=1
        )
        in_maps.append(
            {
                "xT": _mm_np(xp[:, :, nl_0:]),
                "xw": _mm_np(xwp),
                "w1f": _mm_np(w1fp[:, :, _H // 2 :]),
                "w2": _mm_np(W2[p].reshape(_H // 128, 128, _D)),
                "b12": np.ascontiguousarray(b12p),
            }
        )
    return in_maps, C, order, offs, counts


def kernel(latents, actions, policy_indices, W1, b1, W2, b2):
    from concourse.bass_utils import run_bass_kernel_spmd

    in_maps, C, order, offs, counts = _prepare_in_maps(
        latents, actions, policy_indices, W1, b1, W2, b2
    )
    nc = _get_bass(C)
    results = run_bass_kernel_spmd(nc, in_maps, list(range(_N_CORES))).results

    B = np.asarray(latents).shape[0]
    md = _D // 128
    out = np.empty((B, _D), dtype=np.float32)
    for p in range(_P):
        yT = (
            np.asarray(results[p]["outT"], dtype=np.float32)
            .reshape(128, md, C)
            .transpose(1, 0, 2)
            .reshape(_D, C)
        )
        out[order[offs[p] : offs[p + 1]]] = yT[:, : counts[p]].T
    return out
